# Initial kernel scaffold
#
"""Trainium2 Bass kernel for nn_Conv_SelfAttn (1x1-conv QKV + NxN self-attention).

Sharding: data-parallel over batch — 8 batch images, one per NeuronCore.
Each core computes its full 4096x4096 attention locally.

Math per core (b fixed):
  xt = x[b]                                  # [C=128, N=4096]
  qT[o, n] = sum_c Wq[o, c] xt[c, n]         # [16, N]
  kT[o, n] = sum_c Wk[o, c] xt[c, n]         # [16, N]
  v[n, c]  = sum_c' (gamma*Wv)[c, c'] xt[c', n]   # [N, C]  (gamma folded into Wv)
  ET[j, i] = sum_o kT[o, j] qT[o, i]         # energy, stored transposed
  P = exp(ET)                                # no max-subtraction: |E| <~ 30, safe in f32
  num[i, c] = sum_j P[j, i] v[j, c]  ;  den[i] = sum_j P[j, i]   (ones column of v_aug)
  out[c, i] = transpose(num/den) ;  result = xt + out            (gamma already in v)

Layout tricks:
  - qT/kT are built replicated 4x across 32-partition strips (zero rows between)
    so the K=16 energy matmuls can be row-tiled: 2 concurrent matmuls per PE pass.
  - exp tiles [j=128, i] are exactly the stationary (lhsT) operand the PV matmul
    needs — no transposes anywhere in the N^2 path.
  - ones column appended to v gives the softmax denominator from the same matmul.
"""

import numpy as np

B, C, HGT, WID = 8, 128, 64, 64
N = HGT * WID            # 4096
P = 128
NCORES = 8
ICH = 512                # i-chunk (columns of energy per inner pass)
NIC = N // ICH           # 8
NJB = N // P             # 32 j-blocks
VW = 132                 # v_aug allocated width (128 ch + ones col + pad)
NVC = 129                # PV matmul moving width: 128 channels + ones col

_CACHE = {}


def _split_excess_waits(nc, mybir, ctrl_cap=1, compute_cap=8):
    """The pinned walrus accepts only 1 sync-wait on CTRL_NO-class instructions
    (Nop/Drain); Tile's tail drain carries one wait per outstanding processor.
    Hoist excess waits onto preceding single-wait Nops on the same engine."""
    ctrl_types = (mybir.InstNoOp, mybir.InstDrain)
    for f in nc.m.functions:
        for bb in f.blocks:
            new_instructions = []
            for ins in bb.instructions:
                si = ins.sync_info
                waits = list(si.on_wait) if si and si.on_wait else []
                cap = ctrl_cap if isinstance(ins, ctrl_types) else compute_cap
                if len(waits) > cap:
                    keep = waits[:cap] if cap > 0 else []
                    for i, w in enumerate(waits[cap:]):
                        nop = mybir.InstNoOp(
                            name=f"{ins.name}-ws{i}",
                            engine=ins.engine,
                            ins=[],
                            outs=[],
                            sync_info=mybir.SyncInfo(on_wait=[w], on_update=[]),
                        )
                        nc.register_instruction(nop, overwrite=True)
                        new_instructions.append(nop)
                    ins.sync_info = mybir.SyncInfo(
                        on_wait=keep, on_update=list(si.on_update or [])
                    )
                new_instructions.append(ins)
            bb.instructions[:] = new_instructions


def _build():
    import concourse.bass as bass
    import concourse.mybir as mybir
    import concourse.tile as tile
    from concourse.masks import make_identity

    fp32 = mybir.dt.float32
    bf16 = mybir.dt.bfloat16
    Exp = mybir.ActivationFunctionType.Exp
    mult = mybir.AluOpType.mult
    add = mybir.AluOpType.add

    nc = bass.Bass("TRN2", target_bir_lowering=False, debug=False)
    x32 = nc.dram_tensor("x32", [P, N], fp32, kind="ExternalInput")
    x16 = nc.dram_tensor("x16", [P, N], bf16, kind="ExternalInput")
    wqr = nc.dram_tensor("wqr", [P, P], bf16, kind="ExternalInput")
    wkr = nc.dram_tensor("wkr", [P, P], bf16, kind="ExternalInput")
    wvt = nc.dram_tensor("wvt", [P, P], bf16, kind="ExternalInput")
    out = nc.dram_tensor("out", [P, N], fp32, kind="ExternalOutput")

    with tile.TileContext(nc) as tc:
        with (
            tc.tile_pool(name="const", bufs=1) as cp,
            tc.tile_pool(name="pt", bufs=3) as ptp,
            tc.tile_pool(name="work", bufs=2) as wp,
            tc.tile_pool(name="pe", bufs=2, space="PSUM") as pep,
            tc.tile_pool(name="po", bufs=2, space="PSUM") as pop,
            tc.tile_pool(name="ptr", bufs=2, space="PSUM") as trp,
        ):
            xf = cp.tile([P, N], fp32, tag="xf")
            xb = cp.tile([P, N], bf16, tag="xb")
            wq_t = cp.tile([P, P], bf16, tag="wq")
            wk_t = cp.tile([P, P], bf16, tag="wk")
            wv_t = cp.tile([P, P], bf16, tag="wv")
            ident = cp.tile([P, P], fp32, tag="id")
            qrep = cp.tile([P, N], bf16, tag="qrep")
            krep = cp.tile([P, N], bf16, tag="krep")
            vaug = cp.tile([P, NJB, VW], bf16, tag="vaug")

            nc.sync.dma_start(xb[:], x16.ap())
            nc.sync.dma_start(wq_t[:], wqr.ap())
            nc.sync.dma_start(wk_t[:], wkr.ap())
            nc.sync.dma_start(wv_t[:], wvt.ap())
            nc.sync.dma_start(xf[:], x32.ap())
            make_identity(nc, ident[:])
            nc.vector.memset(vaug[:, :, 128:VW], 1.0)

            # --- projections ---
            for ic in range(NIC):
                sl = slice(ic * ICH, (ic + 1) * ICH)
                pq = pep.tile([P, 1024], fp32, tag="e")
                nc.tensor.matmul(
                    pq[:, 0:ICH], wq_t[:], xb[:, sl], start=True, stop=True
                )
                nc.vector.tensor_copy(qrep[:, sl], pq[:, 0:ICH])
                pk = pep.tile([P, 1024], fp32, tag="e")
                nc.tensor.matmul(
                    pk[:, 0:ICH], wk_t[:], xb[:, sl], start=True, stop=True
                )
                nc.vector.tensor_copy(krep[:, sl], pk[:, 0:ICH])
            for jb in range(NJB):
                pv = pep.tile([P, 1024], fp32, tag="e")
                nc.tensor.matmul(
                    pv[:, 0:P],
                    xb[:, jb * P : (jb + 1) * P],
                    wv_t[:],
                    start=True,
                    stop=True,
                )
                nc.vector.tensor_copy(vaug[:, jb, 0:P], pv[:, 0:P])

            # --- main loop ---
            for ic in range(NIC):
                isl = slice(ic * ICH, (ic + 1) * ICH)
                po = [
                    pop.tile([P, 2 * NVC], fp32, tag="o"),
                    pop.tile([P, 2 * NVC], fp32, tag="o"),
                ]
                for g in range(NJB // 2):
                    jb0, jb1 = 2 * g, 2 * g + 1
                    e = pep.tile([P, 1024], fp32, tag="e")
                    nc.tensor.matmul(
                        e[:, 0:512],
                        krep[0:32, jb0 * P : (jb0 + 1) * P],
                        qrep[0:32, isl],
                        start=True,
                        stop=True,
                        tile_position=(0, 0),
                    )
                    nc.tensor.matmul(
                        e[:, 512:1024],
                        krep[32:64, jb1 * P : (jb1 + 1) * P],
                        qrep[32:64, isl],
                        start=True,
                        stop=True,
                        tile_position=(32, 0),
                    )
                    pt = ptp.tile([P, 1024], bf16, tag="pt")
                    nc.scalar.activation(pt[:], e[:], Exp)
                    for h, jb in ((0, jb0), (1, jb1)):
                        for s in range(4):
                            nc.tensor.matmul(
                                po[s // 2][:, NVC * (s % 2) : NVC * (s % 2) + NVC],
                                pt[:, h * 512 + s * P : h * 512 + (s + 1) * P],
                                vaug[:, jb, 0:NVC],
                                start=(g == 0 and h == 0),
                                stop=(g == NJB // 2 - 1 and h == 1),
                                skip_group_check=True,
                            )

                # normalize, transpose back to [c, i], residual-add, store
                tp = trp.tile([P, ICH], fp32, tag="t")
                onorm = wp.tile([P, ICH], fp32, tag="onorm")
                rcp = wp.tile([P, 4], fp32, tag="rcp")
                for s in range(4):
                    src = po[s // 2][:, NVC * (s % 2) : NVC * (s % 2) + NVC]
                    nc.vector.reciprocal(rcp[:, s : s + 1], src[:, 128:129])
                    nc.vector.tensor_scalar(
                        onorm[:, s * P : (s + 1) * P],
                        src[:, 0:P],
                        rcp[:, s : s + 1],
                        None,
                        mult,
                    )
                    nc.tensor.transpose(
                        tp[:, s * P : (s + 1) * P],
                        onorm[:, s * P : (s + 1) * P],
                        ident[:],
                    )
                res = wp.tile([P, ICH], fp32, tag="res")
                nc.vector.tensor_tensor(res[:], tp[:], xf[:, isl], add)
                nc.sync.dma_start(out.ap()[:, isl], res[:])

    _split_excess_waits(nc, mybir)
    return nc


def kernel(x, Wq, Wk, Wv, gamma):
    import ml_dtypes
    from concourse.bass_utils import run_bass_kernel_spmd

    if "nc" not in _CACHE:
        _CACHE["nc"] = _build()
    nc = _CACHE["nc"]

    x = np.asarray(x)
    gamma_v = float(np.asarray(gamma).reshape(-1)[0])
    xt = x.reshape(B, C, N).astype(np.float32)

    def rep_pattern(w):
        # [128, 128]: cols 32t..32t+16 = w.T, else zero
        m = np.zeros((P, P), np.float32)
        for t in range(4):
            m[:, 32 * t : 32 * t + 16] = w.T
        return m.astype(ml_dtypes.bfloat16)

    wq_rep = rep_pattern(np.asarray(Wq, np.float32))
    wk_rep = rep_pattern(np.asarray(Wk, np.float32))
    wv_t = np.ascontiguousarray((gamma_v * np.asarray(Wv, np.float32)).T).astype(
        ml_dtypes.bfloat16
    )

    in_maps = []
    for core in range(NCORES):
        in_maps.append(
            {
                "x32": np.ascontiguousarray(xt[core]),
                "x16": np.ascontiguousarray(xt[core]).astype(ml_dtypes.bfloat16),
                "wqr": wq_rep,
                "wkr": wk_rep,
                "wvt": wv_t,
            }
        )

    res = run_bass_kernel_spmd(nc, in_maps, core_ids=list(range(NCORES)))
    _CACHE["last_results"] = res
    full = np.stack([res.results[core]["out"] for core in range(NCORES)])
    return full.reshape(B, C, HGT, WID).astype(np.float32)


# revision 7
# speedup vs baseline: 2.2204x; 2.2204x over previous
"""Trainium2 Bass kernel for nn_Conv_SelfAttn (1x1-conv QKV + NxN self-attention).

Sharding: data-parallel over batch — 8 batch images, one per NeuronCore.
Each core computes its full 4096x4096 attention locally.

Math per core (b fixed):
  xt = x[b]                                  # [C=128, N=4096]
  qT[o, n] = sum_c Wq[o, c] xt[c, n]         # [16, N]
  kT[o, n] = sum_c Wk[o, c] xt[c, n]         # [16, N]
  v[n, c]  = sum_c' (gamma*Wv)[c, c'] xt[c', n]   # [N, C]  (gamma folded into Wv)
  ET[j, i] = sum_o kT[o, j] qT[o, i]         # energy, stored transposed
  P = exp(ET)                                # no max-subtraction: |E| <~ 30, safe in f32
  num[i, c] = sum_j P[j, i] v[j, c]  ;  den[i] = sum_j P[j, i]   (ones column of v_aug)
  out[c, i] = transpose(num/den) ;  result = xt + out            (gamma already in v)

Layout tricks:
  - qT/kT are built replicated 4x across 32-partition strips (zero rows between)
    so the K=16 energy matmuls can be row-tiled: 2 concurrent matmuls per PE pass.
  - exp tiles [j=128, i] are exactly the stationary (lhsT) operand the PV matmul
    needs — no transposes anywhere in the N^2 path.
  - ones column appended to v gives the softmax denominator from the same matmul.
"""

import numpy as np

B, C, HGT, WID = 8, 128, 64, 64
N = HGT * WID            # 4096
P = 128
NCORES = 8
ICH = 512                # i-chunk (columns of energy per inner pass)
NIC = N // ICH           # 8
NJB = N // P             # 32 j-blocks
VW = 132                 # v_aug allocated width (128 ch + ones col + pad)
NVC = 129                # PV matmul moving width: 128 channels + ones col

_CACHE = {}


def _split_excess_waits(nc, mybir, ctrl_cap=1, compute_cap=1):
    """The pinned walrus accepts only 1 sync-wait on CTRL_NO-class instructions
    (Nop/Drain); Tile's tail drain carries one wait per outstanding processor.
    Hoist excess waits onto preceding single-wait Nops on the same engine."""
    ctrl_types = (mybir.InstNoOp, mybir.InstDrain)
    for f in nc.m.functions:
        for bb in f.blocks:
            new_instructions = []
            for ins in bb.instructions:
                si = ins.sync_info
                waits = list(si.on_wait) if si and si.on_wait else []
                cap = ctrl_cap if isinstance(ins, ctrl_types) else compute_cap
                if len(waits) > cap:
                    keep = waits[:cap] if cap > 0 else []
                    for i, w in enumerate(waits[cap:]):
                        nop = mybir.InstNoOp(
                            name=f"{ins.name}-ws{i}",
                            engine=ins.engine,
                            ins=[],
                            outs=[],
                            sync_info=mybir.SyncInfo(on_wait=[w], on_update=[]),
                        )
                        nc.register_instruction(nop, overwrite=True)
                        new_instructions.append(nop)
                    ins.sync_info = mybir.SyncInfo(
                        on_wait=keep, on_update=list(si.on_update or [])
                    )
                new_instructions.append(ins)
            bb.instructions[:] = new_instructions


def _build(reps=1):
    import contextlib
    import concourse.bass as bass
    import concourse.mybir as mybir
    import concourse.tile as tile
    from concourse.masks import make_identity

    fp32 = mybir.dt.float32
    bf16 = mybir.dt.bfloat16
    Exp = mybir.ActivationFunctionType.Exp
    mult = mybir.AluOpType.mult
    add = mybir.AluOpType.add

    nc = bass.Bass("TRN2", target_bir_lowering=False, debug=False)
    x32 = nc.dram_tensor("x32", [P, N], fp32, kind="ExternalInput")
    x16 = nc.dram_tensor("x16", [P, N], bf16, kind="ExternalInput")
    wqr = nc.dram_tensor("wqr", [P, P], bf16, kind="ExternalInput")
    wkr = nc.dram_tensor("wkr", [P, P], bf16, kind="ExternalInput")
    wvt = nc.dram_tensor("wvt", [P, P], bf16, kind="ExternalInput")
    out = nc.dram_tensor("out", [P, N], fp32, kind="ExternalOutput")

    with tile.TileContext(nc) as tc:
        with (
            tc.tile_pool(name="const", bufs=1) as cp,
            tc.tile_pool(name="pt", bufs=3) as ptp,
            tc.tile_pool(name="work", bufs=2) as wp,
            tc.tile_pool(name="pe", bufs=2, space="PSUM") as pep,
            tc.tile_pool(name="po", bufs=2, space="PSUM") as pop,
            tc.tile_pool(name="ptr", bufs=2, space="PSUM") as trp,
        ):
            xf = cp.tile([P, N], fp32, tag="xf")
            xb = cp.tile([P, N], bf16, tag="xb")
            wq_t = cp.tile([P, P], bf16, tag="wq")
            wk_t = cp.tile([P, P], bf16, tag="wk")
            wv_t = cp.tile([P, P], bf16, tag="wv")
            ident = cp.tile([P, P], fp32, tag="id")
            qrep = cp.tile([P, N], bf16, tag="qrep")
            krep = cp.tile([P, N], bf16, tag="krep")
            vaug = cp.tile([P, NJB, VW], bf16, tag="vaug")

            nc.sync.dma_start(xb[:], x16.ap())
            nc.sync.dma_start(wq_t[:], wqr.ap())
            nc.sync.dma_start(wk_t[:], wkr.ap())
            nc.sync.dma_start(wv_t[:], wvt.ap())
            nc.sync.dma_start(xf[:], x32.ap())
            make_identity(nc, ident[:])
            nc.vector.memset(vaug[:, :, 128:VW], 1.0)

            # reps>1 wraps the whole compute in a HW loop — benchmarking only
            loop_ctx = tc.For_i(0, reps, 1) if reps > 1 else contextlib.nullcontext()
            with loop_ctx:
                _emit_body(nc, mybir, qrep, krep, vaug, xb, xf, wq_t, wk_t, wv_t,
                           ident, out, pep, pop, trp, ptp, wp)

    _split_excess_waits(nc, mybir)
    return nc


def _emit_body(nc, mybir, qrep, krep, vaug, xb, xf, wq_t, wk_t, wv_t, ident, out,
               pep, pop, trp, ptp, wp):
    fp32 = mybir.dt.float32
    bf16 = mybir.dt.bfloat16
    Exp = mybir.ActivationFunctionType.Exp
    mult = mybir.AluOpType.mult
    add = mybir.AluOpType.add
    if True:
        if True:
            # --- projections ---
            for ic in range(NIC):
                sl = slice(ic * ICH, (ic + 1) * ICH)
                pq = pep.tile([P, 1024], fp32, tag="e")
                nc.tensor.matmul(
                    pq[:, 0:ICH], wq_t[:], xb[:, sl], start=True, stop=True
                )
                nc.vector.tensor_copy(qrep[:, sl], pq[:, 0:ICH])
                pk = pep.tile([P, 1024], fp32, tag="e")
                nc.tensor.matmul(
                    pk[:, 0:ICH], wk_t[:], xb[:, sl], start=True, stop=True
                )
                nc.vector.tensor_copy(krep[:, sl], pk[:, 0:ICH])
            for jb in range(NJB):
                pv = pep.tile([P, 1024], fp32, tag="e")
                nc.tensor.matmul(
                    pv[:, 0:P],
                    xb[:, jb * P : (jb + 1) * P],
                    wv_t[:],
                    start=True,
                    stop=True,
                )
                nc.vector.tensor_copy(vaug[:, jb, 0:P], pv[:, 0:P])

            # --- main loop ---
            for ic in range(NIC):
                isl = slice(ic * ICH, (ic + 1) * ICH)
                po = [
                    pop.tile([P, 2 * NVC], fp32, tag="o", name=f"po_a_{ic}"),
                    pop.tile([P, 2 * NVC], fp32, tag="o", name=f"po_b_{ic}"),
                ]
                for g in range(NJB // 2):
                    jb0, jb1 = 2 * g, 2 * g + 1
                    e = pep.tile([P, 1024], fp32, tag="e")
                    nc.tensor.matmul(
                        e[:, 0:512],
                        krep[0:32, jb0 * P : (jb0 + 1) * P],
                        qrep[0:32, isl],
                        start=True,
                        stop=True,
                        tile_position=(0, 0),
                    )
                    nc.tensor.matmul(
                        e[:, 512:1024],
                        krep[32:64, jb1 * P : (jb1 + 1) * P],
                        qrep[32:64, isl],
                        start=True,
                        stop=True,
                        tile_position=(32, 0),
                    )
                    pt = ptp.tile([P, 1024], bf16, tag="pt")
                    nc.scalar.activation(pt[:], e[:], Exp)
                    for h, jb in ((0, jb0), (1, jb1)):
                        for s in range(4):
                            nc.tensor.matmul(
                                po[s // 2][:, NVC * (s % 2) : NVC * (s % 2) + NVC],
                                pt[:, h * 512 + s * P : h * 512 + (s + 1) * P],
                                vaug[:, jb, 0:NVC],
                                # start=True clears has_written for the WHOLE
                                # bank — only the first matmul into each po
                                # bank may set it; the other region's first
                                # write lands on cleared bits and overwrites.
                                start=(g == 0 and h == 0 and s % 2 == 0),
                                stop=(g == NJB // 2 - 1 and h == 1),
                                skip_group_check=True,
                            )

                # normalize, transpose back to [c, i], residual-add, store
                tp = trp.tile([P, ICH], fp32, tag="t")
                onorm = wp.tile([P, ICH], fp32, tag="onorm")
                rcp = wp.tile([P, 4], fp32, tag="rcp")
                for s in range(4):
                    src = po[s // 2][:, NVC * (s % 2) : NVC * (s % 2) + NVC]
                    nc.vector.reciprocal(rcp[:, s : s + 1], src[:, 128:129])
                    nc.vector.tensor_scalar(
                        onorm[:, s * P : (s + 1) * P],
                        src[:, 0:P],
                        rcp[:, s : s + 1],
                        None,
                        mult,
                    )
                    nc.tensor.transpose(
                        tp[:, s * P : (s + 1) * P],
                        onorm[:, s * P : (s + 1) * P],
                        ident[:],
                    )
                res = wp.tile([P, ICH], fp32, tag="res")
                nc.vector.tensor_tensor(res[:], tp[:], xf[:, isl], add)
                nc.sync.dma_start(out.ap()[:, isl], res[:])


def kernel(x, Wq, Wk, Wv, gamma):
    import ml_dtypes
    from concourse.bass_utils import run_bass_kernel_spmd

    if "nc" not in _CACHE:
        _CACHE["nc"] = _build()
    nc = _CACHE["nc"]

    x = np.asarray(x)
    gamma_v = float(np.asarray(gamma).reshape(-1)[0])
    xt = x.reshape(B, C, N).astype(np.float32)

    def rep_pattern(w):
        # [128, 128]: cols 32t..32t+16 = w.T, else zero
        m = np.zeros((P, P), np.float32)
        for t in range(4):
            m[:, 32 * t : 32 * t + 16] = w.T
        return m.astype(ml_dtypes.bfloat16)

    wq_rep = rep_pattern(np.asarray(Wq, np.float32))
    wk_rep = rep_pattern(np.asarray(Wk, np.float32))
    wv_t = np.ascontiguousarray((gamma_v * np.asarray(Wv, np.float32)).T).astype(
        ml_dtypes.bfloat16
    )

    in_maps = []
    for core in range(NCORES):
        in_maps.append(
            {
                "x32": np.ascontiguousarray(xt[core]),
                "x16": np.ascontiguousarray(xt[core]).astype(ml_dtypes.bfloat16),
                "wqr": wq_rep,
                "wkr": wk_rep,
                "wvt": wv_t,
            }
        )

    res = run_bass_kernel_spmd(nc, in_maps, core_ids=list(range(NCORES)))
    _CACHE["last_results"] = res
    full = np.stack([res.results[core]["out"] for core in range(NCORES)])
    return full.reshape(B, C, HGT, WID).astype(np.float32)
